# revision 56
# baseline (speedup 1.0000x reference)
"""Trainium2 Bass kernel for nn_AdaptiveConditionedGraphTransformer.

Strategy (8 NeuronCores, data-parallel over nodes, 128 nodes/core):
- Graph edge-attention (PyG TransformerConv) computed DENSELY per core as
  [src=1024, dst=128] score matrices on TensorE; duplicate edges + masking
  handled exactly via a host-precomputed ln(count) additive mask injected
  into PSUM with an identity matmul inside the score accumulation group.
- Layer-0 tconv (fin=10) uses a rank-11 factorization: S = x_aug A x_aug^T
  with A precomputed on host; all 4 heads' score matmuls fused into one
  N=512 instruction per src chunk. Only a 5.5KB AllGather of x per step.
- Layer-1 tconv scores use the same bilinear trick at rank 257:
  S_h = [mid,1] @ (Wq_aug_h Wk_h^T/16) @ mid_all^T, so no Q/K projections
  exist at all. ONE fp8 collective per step gathers mid in both layouts
  (midT for scores + node-major for aggregation, 64KB in); in the gen
  phase it is split in two so the node-major half lands under the score
  phase. No V gather: the attention aggregates the 256-dim mid directly
  (Pm_h = sum_src P_h mid_src), then projects Pm_h through Wv1_h after
  the softmax-weighted sum (transpose + per-head matmul); bv1's head-mean
  folds into the br1+pe constant since softmax weights sum to 1. M1 =
  Ahat^T @ [midT;1] is computed locally during the gather.
- Softmax denominators ride along in the aggregation matmuls (ones column
  appended to gathered mid / e10 column appended to Wv0aug) -- no separate
  z matmuls.
- Temporal transformer layer: exact KV-cache, last-query-only; q/k/v cache
  projections fused into one PE burst after tconv1; all-head score and
  output reductions as single strided multi-dim DVE ops; softmax 1/z
  applied on the 64-wide head outputs.
- All biases folded into broadcast-tile adds on the PSUM->SBUF copies or
  per-partition scalar ops -- no ones-row bias matmuls. FFN relu+bias and
  half the PSUM->SBUF staging copies run on ScalarE (relu/copy share the
  exp activation table set, so no table switches). Per-chunk exp staging
  tiles are allocated inside the chunk loops so the EXP(Scalar) ->
  mask-mult(DVE) chains double-buffer instead of serializing.
- LayerNorm rsqrt via bit-trick + one Newton step on DVE; ScalarE runs only
  Exp (single activation table set, no ~2.7us table switches).
- Matmul inputs fp16 (fp32 PSUM); fp8 e4m3 only on softmax/mean-protected
  paths (gathered midT + M1 for scores, layer-1 probs, gathered V), enabling
  DoubleRow (2 K-chunks/instruction) on the layer-1 score and aggregation
  matmuls; everything else f16 to hold rel_err ~4.5e-3.

kernel(**inputs) takes FULL inputs, shards internally, returns FULL output.
"""
import os
import sys

import numpy as np

sys.path.insert(0, "/opt/trn_rl_repo")

N, E, F = 1024, 16384, 10
DM, H = 256, 4
D = 256
NG, K = 20, 10
FF = 2048
DH = DM // H
NC_ = 8
P = N // NC_          # 128 nodes per core
AUG = F + 1           # 11
TCACHE = NG - 1       # 19 cache slots

N_KNOWN = int(os.environ.get("GT_KNOWN", "10"))
N_GEN = int(os.environ.get("GT_GEN", "10"))

_NEG = -30000.0


# ----------------------------------------------------------------------------
# Device program
# ----------------------------------------------------------------------------
def build_bass():
    import concourse.bass as bass
    import concourse.tile as tile
    from concourse import bacc, mybir

    f16 = mybir.dt.float16
    f32 = mybir.dt.float32
    f8 = mybir.dt.float8e4
    bf16 = mybir.dt.bfloat16
    AF = mybir.ActivationFunctionType
    OP = mybir.AluOpType
    AX = mybir.AxisListType

    nc = bacc.Bacc("TRN2", target_bir_lowering=False, debug=False, num_devices=NC_)
    RG = [list(range(NC_))]

    def din(name, shape, dtype):
        return nc.dram_tensor(name, list(shape), dtype, kind="ExternalInput").ap()

    d = {}
    d["lt"] = din("lt", [P, 8, P], f16)
    d["cnt"] = din("cnt", [P, 8, P], f16)
    d["latlon32"] = din("latlon32", [P, 2], f32)
    d["AT"] = din("AT", [AUG, H, AUG], f16)
    d["Ahat18"] = din("Ahat18", [P, 2, H, DM], f8)
    d["Wv0az"] = din("Wv0az", [AUG, H, D + 1], f16)
    d["Wr0a"] = din("Wr0a", [AUG, DM], f16)
    d["kxTaug_all"] = din("kxTaug_all", [K, AUG, N], f16)
    d["kxaug_all"] = din("kxaug_all", [K, P, 8, 32], f8)
    d["kxTaug_loc"] = din("kxTaug_loc", [K, AUG, P], f16)
    d["Ahat1"] = din("Ahat1", [P, 2, H, DM], f16)
    d["AhatBt"] = din("AhatBt", [P, 2, H], f16)
    d["Wv1"] = din("Wv1", [P, 2, H * D], f16)
    d["Wr1"] = din("Wr1", [P, 2, DM], f16)
    d["brpe1"] = din("brpe1", [NG, 1, DM], f16)
    d["Wqe"] = din("Wqe", [P, 2, DM], f16)
    d["Wke"] = din("Wke", [P, 2, DM], f16)
    d["Wve"] = din("Wve", [P, 2, DM], f16)
    d["Wo"] = din("Wo", [P, 2, DM], f16)
    d["bqe"] = din("bqe", [1, DM], f16)
    d["bke"] = din("bke", [1, DM], f16)
    d["bve"] = din("bve", [1, DM], f16)
    d["bo"] = din("bo", [1, DM], f16)
    d["W1"] = din("W1", [P, 2, FF], f16)
    d["b1f"] = din("b1f", [1, FF], f16)
    d["b1fc"] = din("b1fc", [P, 16], f32)
    d["W2"] = din("W2", [P, 16, DM], f16)
    d["b2f"] = din("b2f", [1, DM], f16)
    d["Wd"] = din("Wd", [P, 2, F - 2], f16)
    d["W2d"] = din("W2d", [P, 16, F - 2], f16)
    d["c1r"] = din("c1r", [1, F - 2], f16)
    d["nsB1"] = din("nsB1", [1, F - 2], f32)
    d["bdB1"] = din("bdB1", [1, F - 2], f32)
    d["bd"] = din("bd", [1, F - 2], f16)
    d["g1"] = din("g1", [1, DM], f32)
    d["be1"] = din("be1", [1, DM], f32)
    d["g2"] = din("g2", [1, DM], f32)
    d["be2"] = din("be2", [1, DM], f32)
    d["xTaug_init"] = din("xTaug_init", [AUG, P], f16)
    d["xaug_init"] = din("xaug_init", [P, AUG], f16)
    d["idn"] = din("idn", [P, P], f16)
    d["ones1"] = din("ones1", [1, P], f16)

    out_d = nc.dram_tensor("out", [N_GEN, P, F], f32, kind="ExternalOutput").ap()

    mm = nc.tensor.matmul
    DR = mybir.MatmulPerfMode.DoubleRow
    V = nc.vector
    S = nc.scalar
    G = nc.gpsimd

    def bc(ap, ins_size):
        """Insert a stride-0 dim before the last free dim: [p, n] -> [p, m, n]."""
        return bass.AP(tensor=ap.tensor, offset=ap.offset,
                       ap=[list(ap.ap[0]), [0, ins_size], list(ap.ap[1])])

    from contextlib import ExitStack
    with tile.TileContext(nc) as tc, ExitStack() as ctx:
        consts = ctx.enter_context(tc.tile_pool(name="consts", bufs=1))
        state = ctx.enter_context(tc.tile_pool(name="state", bufs=1))
        work = ctx.enter_context(tc.tile_pool(name="work", bufs=2))
        big = ctx.enter_context(tc.tile_pool(name="big", bufs=1))
        psA = ctx.enter_context(tc.tile_pool(name="psA", bufs=2, space="PSUM"))
        psS = ctx.enter_context(tc.tile_pool(name="psS", bufs=2, space="PSUM"))
        psM = ctx.enter_context(tc.tile_pool(name="psM", bufs=2, space="PSUM"))
        dram = ctx.enter_context(tc.tile_pool(name="dram", bufs=2, space="DRAM"))

        def ps_big(name, shape=(P, 8, P)):
            return psA.tile(list(shape), f32, name=name, tag="SP")

        def ps_m(shape, name, dtype=None):
            return psM.tile(list(shape), dtype or f32, name=name, tag="mP")

        cs = {}
        _skip = ("nsB1", "bdB1", "g1", "be1", "g2", "be2", "kxTaug_all", "kxaug_all",
                 "kxTaug_loc", "xTaug_init", "xaug_init", "latlon32", "brpe1")
        _early = ("AT", "idn", "ones1", "lt", "cnt", "Wv0az", "Wr0a", "Ahat1",
                  "AhatBt", "Wv1", "Wr1")
        _order = [n for n in _early if n in d] + \
                 [n for n in d if n not in _early and n not in _skip]
        _eng = [nc.sync, nc.scalar, nc.sync, nc.scalar]
        for _i, name in enumerate(_order):
            ap = d[name]
            t = consts.tile(list(ap.shape), ap.dtype, name=f"c_{name}")
            _eng[_i % 4].dma_start(out=t, in_=ap)
            cs[name] = t
        for name in ("g1", "be1", "g2", "be2"):
            t = consts.tile([P, DM], f32, name=f"c_{name}")
            nc.sync.dma_start(out=t, in_=d[name].to_broadcast((P, DM)))
            cs[name] = t
        for name in ("nsB1", "bdB1"):
            t = consts.tile([P, F - 2], f32, name=f"c_{name}b")
            nc.scalar.dma_start(out=t, in_=d[name].to_broadcast((P, F - 2)))
            cs[name + "b"] = t
        for nm, src_nm, w in (("bkeB", "bke", DM), ("bveB", "bve", DM),
                              ("bqeB", "bqe", DM)):
            t = consts.tile([P, w], f16, name=f"c_{nm}")
            nc.sync.dma_start(out=t, in_=d[src_nm].to_broadcast((P, w)))
            cs[nm] = t
        t = consts.tile([P, 16], f32, name="c_b1fc")
        nc.sync.dma_start(out=t, in_=d["b1fc"])
        cs["b1fc"] = t
        t = consts.tile([1, NG * DM], f16, name="c_brpeC")
        nc.scalar.dma_start(out=t, in_=d["brpe1"].rearrange("g o m -> o (g m)"))
        cs["brpeC"] = t
        idn = cs["idn"]; ones1 = cs["ones1"]
        lt = cs["lt"]; cnt = cs["cnt"]

        Kc = state.tile([P, TCACHE, DM], f16, name="Kc")
        Vc2 = state.tile([P, H, DH, TCACHE], f16, name="Vc2")
        xTaug = state.tile([AUG, P], f16, name="xTaug")
        xaug = state.tile([P, AUG], f16, name="xaug")
        xa8_st = state.tile([P, 8, 32], f8, name="xa8_st")
        V.memset(xa8_st[:, :, AUG:32], 0.0)
        nc.sync.dma_start(out=xTaug, in_=d["xTaug_init"])
        nc.sync.dma_start(out=xaug, in_=d["xaug_init"])

        # ------------------------------------------------------------------
        def transpose128(src_ap, n_chunks, name, also_f8=False):
            dst = work.tile([P, n_chunks, P], f16, name=name,
                            bufs=(5 if name == "midT" else None))
            dst8 = (work.tile([P, n_chunks, P], f8, name=name + "8", bufs=4)
                    if also_f8 else None)
            for fc in range(n_chunks):
                pt = ps_m([P, P], "ptp", dtype=f16)
                mm(pt, src_ap[:, fc * P:(fc + 1) * P], idn, start=True, stop=True,
                   is_transpose=True)
                V.tensor_copy(out=dst[:, fc, :], in_=pt)
                if also_f8:
                    V.tensor_copy(out=dst8[:, fc, :], in_=pt)
            if also_f8:
                return dst, dst8
            return dst

        def combine(agg01, agg23, rootP, name):
            """sum_h agg_h/(4 z_h) + rootP; z_h rides in agg col 256.
            Head-pair 0/1 is scaled with its own zi tiles so that work can
            start as soon as agg01 stops, overlapping head-2/3 matmuls.
            All zi ops are whole-tile, never in-place (slice-in-place
            reciprocals raced and produced NaN)."""
            zi01 = work.tile([P, 2], f32, name=f"zi01_{name}")
            V.tensor_scalar(out=zi01,
                            in0=agg01[:, :, 256:257].rearrange("p h x -> p (h x)"),
                            scalar1=4.0, scalar2=4e-16, op0=OP.mult, op1=OP.add)
            zr01 = work.tile([P, 2], f32, name=f"zr01_{name}")
            V.reciprocal(out=zr01, in_=zi01)
            t0 = work.tile([P, DM], f32, name=f"cmb_{name}")
            V.tensor_scalar_mul(out=t0, in0=agg01[:, 0, 0:256],
                                scalar1=zr01[:, 0:1])
            V.scalar_tensor_tensor(out=t0, in0=agg01[:, 1, 0:256],
                                   scalar=zr01[:, 1:2], in1=t0,
                                   op0=OP.mult, op1=OP.add)
            zi23 = work.tile([P, 2], f32, name=f"zi23_{name}")
            V.tensor_scalar(out=zi23,
                            in0=agg23[:, :, 256:257].rearrange("p h x -> p (h x)"),
                            scalar1=4.0, scalar2=4e-16, op0=OP.mult, op1=OP.add)
            zr23 = work.tile([P, 2], f32, name=f"zr23_{name}")
            V.reciprocal(out=zr23, in_=zi23)
            V.scalar_tensor_tensor(out=t0, in0=agg23[:, 0, 0:256],
                                   scalar=zr23[:, 0:1], in1=t0,
                                   op0=OP.mult, op1=OP.add)
            V.scalar_tensor_tensor(out=t0, in0=agg23[:, 1, 0:256],
                                   scalar=zr23[:, 1:2], in1=t0,
                                   op0=OP.mult, op1=OP.add)
            out_nm = work.tile([P, DM], f16, name=f"nm_{name}")
            V.tensor_tensor(out=out_nm, in0=rootP, in1=t0, op=OP.add)
            return out_nm

        def tconv0(xTa_all_fn, xa8, xTa_loc, mask_v=False):
            """Rank-11 layer-0 tconv."""
            MpP = ps_m([AUG, H, P], "MpP")
            for h in range(H):
                mm(MpP[:, h, :], cs["AT"][:, h, :], xTa_loc, start=True, stop=True)
            Mp = work.tile([AUG, H, P], f16, name="Mp")
            V.tensor_copy(out=Mp, in_=MpP)
            rootP = ps_m([P, DM], "rootP")
            mm(rootP, xTa_loc, cs["Wr0a"], start=True, stop=True)
            PT = big.tile([P, 8, H, P], f8, name="PT0", tag="PT0", bufs=2)
            GTP = ps_m([32, H, P], "GTP32")
            assert mask_v
            # Chunk-paired scores: one Exp and one cnt-mask multiply cover two
            # src chunks, amortizing per-op PSUM-access/dispatch overhead. The
            # paired [P, 2, H, P] psum reuses the 4KB "SP" slots.
            for cp in range(4):
                sp2 = psA.tile([P, 2, H, P], f32, name="sp2", tag="SP")
                for j in range(2):
                    mm(sp2[:, j, :, :], xTa_all_fn(2 * cp + j), Mp,
                       start=True, stop=True)
                PTe0 = work.tile([P, 2, H, P], f16, name="PTe0", bufs=3)
                S.activation(out=PTe0, in_=sp2, func=AF.Exp)
                cv = cnt[:, 2 * cp:2 * cp + 2, :]
                cnt2 = bass.AP(tensor=cv.tensor, offset=cv.offset,
                               ap=[list(cv.ap[0]), list(cv.ap[1]), [0, H],
                                   list(cv.ap[2])])
                V.tensor_tensor(out=PT[:, 2 * cp:2 * cp + 2, :, :], in0=PTe0,
                                in1=cnt2, op=OP.mult)
            for cp in range(4):
                mm(GTP, xa8[:, 2 * cp:2 * cp + 2, :],
                   PT[:, 2 * cp:2 * cp + 2, :, :],
                   start=(cp == 0), stop=(cp == 3), perf_mode=DR)
            GT = work.tile([AUG, H, P], f16, name="GT")
            V.tensor_copy(out=GT, in_=GTP[0:AUG, :, :])
            agg01 = ps_big("agg01", (P, 2, 512))
            agg23 = ps_big("agg23", (P, 2, 512))
            for h in range(H):
                dst = (agg01, agg23)[h // 2][:, h % 2, 0:D + 1]
                mm(dst, GT[:, h, :], cs["Wv0az"][:, h, :], start=True, stop=True)
            mid = combine(agg01, agg23, rootP, "t0")
            m8 = work.tile([P, DM], f8, name="m8", bufs=4)
            S.copy(out=m8, in_=mid)
            midT, midT8 = transpose128(mid, 2, "midT", also_f8=True)
            return midT, midT8, m8

        def tconv1_attn(midT, mT_all, m_all, M1sb, brpe, idx, mask_v=False,
                        rootP_sb=None):
            """Dense edge attention from gathered midT/mid. Aggregates the
            256-dim mid per head (z rides col 256 via the m_all ones col),
            then projects through Wv1_h after the softmax-weighted sum.
            Returns (h_nm, hT)."""
            PT = big.tile([P, 8, H, P], f8, name="PT", tag="PT", bufs=2)
            assert mask_v
            # Chunk-paired scores, mirroring tconv0: one Exp + one cnt-mask
            # multiply per two src chunks; PT is chunk-major so the paired
            # output slice is natural-order.
            for cp in range(4):
                sp2 = psA.tile([P, 2, H, P], f32, name="sp2", tag="SP")
                for j in range(2):
                    mm(sp2[:, j, :, :], mT_all[:, :, 2 * cp + j, :], M1sb,
                       start=True, stop=True, perf_mode=DR)
                PTe = work.tile([P, 2, H, P], f16, name="PTe", bufs=3)
                S.activation(out=PTe, in_=sp2, func=AF.Exp)
                cv = cnt[:, 2 * cp:2 * cp + 2, :]
                cnt2 = bass.AP(tensor=cv.tensor, offset=cv.offset,
                               ap=[list(cv.ap[0]), list(cv.ap[1]), [0, H],
                                   list(cv.ap[2])])
                V.tensor_tensor(out=PT[:, 2 * cp:2 * cp + 2, :, :], in0=PTe,
                                in1=cnt2, op=OP.mult)
            agg01 = ps_big("agg01", (P, 2, 512))
            agg23 = ps_big("agg23", (P, 2, 512))
            for h in range(H):
                dst = (agg01, agg23)[h // 2][:, h % 2, 0:D + 1]
                for cp in range(4):
                    mm(dst, PT[:, 2 * cp:2 * cp + 2, h, :],
                       m_all[:, 2 * cp:2 * cp + 2, 0:D + 1],
                       start=(cp == 0), stop=(cp == 3), perf_mode=DR)
            # 1/(4 z_h) from the ones-column sums.
            zi = work.tile([P, H], f32, name="zi_t1")
            V.tensor_scalar(out=zi[:, 0:2],
                            in0=agg01[:, :, 256:257].rearrange("p h x -> p (h x)"),
                            scalar1=4.0, scalar2=4e-16, op0=OP.mult, op1=OP.add)
            V.tensor_scalar(out=zi[:, 2:4],
                            in0=agg23[:, :, 256:257].rearrange("p h x -> p (h x)"),
                            scalar1=4.0, scalar2=4e-16, op0=OP.mult, op1=OP.add)
            V.reciprocal(out=zi, in_=zi)
            # Pm [dst, f] -> SBUF f16, transpose to [f, dst], project by Wv1_h.
            Pmf = work.tile([P, H, 2, P], f16, name="Pmf")
            V.tensor_copy(out=Pmf[:, 0, :, :],
                          in_=agg01[:, 0, 0:256].rearrange("p (c j) -> p c j", j=P))
            S.copy(out=Pmf[:, 1, :, :],
                   in_=agg01[:, 1, 0:256].rearrange("p (c j) -> p c j", j=P))
            V.tensor_copy(out=Pmf[:, 2, :, :],
                          in_=agg23[:, 0, 0:256].rearrange("p (c j) -> p c j", j=P))
            S.copy(out=Pmf[:, 3, :, :],
                   in_=agg23[:, 1, 0:256].rearrange("p (c j) -> p c j", j=P))
            PmT = work.tile([P, H, 2, P], f16, name="PmT")
            for h in range(H):
                for fc in range(2):
                    pt = ps_m([P, P], "ptp", dtype=f16)
                    mm(pt, Pmf[:, h, fc, :], idn, start=True, stop=True,
                       is_transpose=True)
                    if (h * 2 + fc) % 2 == 0:
                        V.tensor_copy(out=PmT[:, h, fc, :], in_=pt)
                    else:
                        S.copy(out=PmT[:, h, fc, :], in_=pt)
            projP = psA.tile([P, H, DM], f32, name="projP", tag="SP")
            for h in range(H):
                for fc in range(2):
                    mm(projP[:, h, :], PmT[:, h, fc, :],
                       cs["Wv1"][:, fc, h * D:(h + 1) * D],
                       start=(fc == 0), stop=(fc == 1))
            if rootP_sb is None:
                rootP_t = psS.tile([P, H, P], f32, name="rootP", tag="Scc")
                rootP_sb = rootP_t[:, 0:2, :].rearrange("p a b -> p (a b)")
                mm(rootP_sb, ones1, brpe, start=True, stop=False)
                for fc in range(2):
                    mm(rootP_sb, midT[:, fc, :], cs["Wr1"][:, fc, :],
                       start=False, stop=(fc == 1))
            t0 = work.tile([P, DM], f32, name="cmb_t1")
            V.tensor_scalar_mul(out=t0, in0=projP[:, 0, :], scalar1=zi[:, 0:1])
            for h in range(1, H):
                V.scalar_tensor_tensor(out=t0, in0=projP[:, h, :],
                                       scalar=zi[:, h:h + 1], in1=t0,
                                       op0=OP.mult, op1=OP.add)
            h_nm = work.tile([P, DM], f16, name="nm_t1")
            V.tensor_tensor(out=h_nm, in0=rootP_sb, in1=t0, op=OP.add)
            hT = transpose128(h_nm, 2, "hT")
            return h_nm, hT

        qe_st = state.tile([P, DM], f16, name="qe_st")

        def cache_update(slot, hT):
            # k and q first: enc's score chain needs them; v only feeds the
            # later attention-out reduction.
            for W, b, which in ((cs["Wke"], "bkeB", "k"),
                                (cs["Wqe"], "bqeB", "q"),
                                (cs["Wve"], "bveB", "v")):
                pp_t = psS.tile([P, H, P], f32, name="cuP", tag="Scc")
                pp = pp_t[:, 0:2, :].rearrange("p a b -> p (a b)")
                for fc in range(2):
                    mm(pp, hT[:, fc, :], W[:, fc, :], start=(fc == 0),
                       stop=(fc == 1))
                bB = cs[b]
                if which == "k":
                    V.tensor_tensor(out=Kc[:, slot, :], in0=pp, in1=bB, op=OP.add)
                elif which == "v":
                    V.tensor_tensor(out=Vc2[:, :, :, slot],
                                    in0=pp.rearrange("p (h e) -> p h e", h=H),
                                    in1=bB.rearrange("p (h e) -> p h e", h=H),
                                    op=OP.add)
                else:
                    V.tensor_tensor(out=qe_st, in0=pp, in1=bB, op=OP.add)

        def layer_norm(x_f32_psum, resid_f16, g, be, name):
            t1 = work.tile([P, DM], f32, name=f"ln_t1_{name}")
            V.tensor_tensor(out=t1, in0=x_f32_psum, in1=resid_f16, op=OP.add)
            st = work.tile([P, 6], f32, name=f"ln_st_{name}")
            V.bn_stats(out=st, in_=t1)
            mv = work.tile([P, 2], f32, name=f"ln_mv_{name}")
            V.bn_aggr(out=mv, in_=st)
            vv = work.tile([P, 1], f32, name=f"ln_vv_{name}")
            V.tensor_scalar_add(out=vv, in0=mv[:, 1:2], scalar1=1e-5)
            rs = work.tile([P, 1], f32, name=f"ln_rs_{name}")
            rsi = rs.bitcast(mybir.dt.int32)
            V.tensor_scalar(out=rsi, in0=vv.bitcast(mybir.dt.int32),
                            scalar1=1, scalar2=None, op0=OP.arith_shift_right)
            V.tensor_scalar(out=rsi, in0=rsi, scalar1=-1, scalar2=0x5F3759DF,
                            op0=OP.mult, op1=OP.add)
            t_n = work.tile([P, 1], f32, name=f"ln_nt_{name}")
            for _ in range(2):
                V.tensor_tensor(out=t_n, in0=rs, in1=rs, op=OP.mult)
                V.tensor_tensor(out=t_n, in0=t_n, in1=vv, op=OP.mult)
                V.tensor_scalar(out=t_n, in0=t_n, scalar1=-0.5, scalar2=1.5,
                                op0=OP.mult, op1=OP.add)
                V.tensor_tensor(out=rs, in0=rs, in1=t_n, op=OP.mult)
                break
            V.tensor_scalar(out=t1, in0=t1, scalar1=mv[:, 0:1], scalar2=rs,
                            op0=OP.subtract, op1=OP.mult)
            V.tensor_tensor(out=t1, in0=t1, in1=g, op=OP.mult)
            o = work.tile([P, DM], f16, name=f"ln_o_{name}")
            V.tensor_tensor(out=o, in0=t1, in1=be, op=OP.add)
            return o

        def enc(t, h_nm_last):
            # Scores for all heads in two fused DVE ops: Kc viewed [p,h,t,e]
            # (strided) times q broadcast over t, then innermost reduce.
            sc = work.tile([P, H, TCACHE], f16, name="sc")
            tmp = work.tile([P, H, TCACHE, DH], f16, name="sctmp", tag="etmp")
            kcv = Kc[:, 0:t, :]
            kc_htE = bass.AP(tensor=kcv.tensor, offset=kcv.offset,
                             ap=[list(kcv.ap[0]), [DH, H], [DM, t], [1, DH]])
            qv = qe_st[:]
            q_htE = bass.AP(tensor=qv.tensor, offset=qv.offset,
                            ap=[list(qv.ap[0]), [DH, H], [0, t], [1, DH]])
            V.tensor_tensor(out=tmp[:, :, 0:t, :], in0=kc_htE, in1=q_htE,
                            op=OP.mult)
            with nc.allow_low_precision("f16 attn scores, |s|<~4"):
                V.tensor_tensor(out=tmp[:, :, 0:t, 0:DH // 2],
                                in0=tmp[:, :, 0:t, 0:DH // 2],
                                in1=tmp[:, :, 0:t, DH // 2:DH], op=OP.add)
                V.tensor_tensor(out=tmp[:, :, 0:t, 0:DH // 4],
                                in0=tmp[:, :, 0:t, 0:DH // 4],
                                in1=tmp[:, :, 0:t, DH // 4:DH // 2], op=OP.add)
                V.tensor_tensor(out=tmp[:, :, 0:t, 0:DH // 8],
                                in0=tmp[:, :, 0:t, 0:DH // 8],
                                in1=tmp[:, :, 0:t, DH // 8:DH // 4], op=OP.add)
                V.tensor_reduce(out=sc[:, :, 0:t],
                                in_=tmp[:, :, 0:t, 0:DH // 8],
                                axis=AX.X, op=OP.add)
            S.activation(out=sc[:, :, 0:t], in_=sc[:, :, 0:t], func=AF.Exp)
            z = work.tile([P, H], f32, name="ze")
            V.tensor_reduce(out=z, in_=sc[:, :, 0:t], axis=AX.X, op=OP.add)
            V.reciprocal(out=z, in_=z)
            o = work.tile([P, DM], f16, name="oe")
            ow = work.tile([P, H, DH], f32, name="ow")
            tmp2 = work.tile([P, H, DH, TCACHE], f16, name="otmp", tag="etmp")
            scv = sc[:]
            sc_hEt = bass.AP(tensor=scv.tensor, offset=scv.offset,
                             ap=[list(scv.ap[0]), [TCACHE, H], [0, DH], [1, t]])
            V.tensor_tensor(out=tmp2[:, :, :, 0:t], in0=Vc2[:, :, :, 0:t],
                            in1=sc_hEt, op=OP.mult)
            th = (t + 1) // 2
            th2 = (th + 1) // 2
            with nc.allow_low_precision("f16 attn out, probs sum to 1"):
                V.tensor_tensor(out=tmp2[:, :, :, 0:t // 2],
                                in0=tmp2[:, :, :, 0:t // 2],
                                in1=tmp2[:, :, :, th:th + t // 2], op=OP.add)
                V.tensor_tensor(out=tmp2[:, :, :, 0:th // 2],
                                in0=tmp2[:, :, :, 0:th // 2],
                                in1=tmp2[:, :, :, th2:th2 + th // 2], op=OP.add)
                V.tensor_reduce(out=ow, in_=tmp2[:, :, :, 0:th2], axis=AX.X,
                                op=OP.add)
            for h in range(H):
                V.tensor_scalar_mul(out=o[:, h * DH:(h + 1) * DH],
                                    in0=ow[:, h, :], scalar1=z[:, h:h + 1])
            oT = transpose128(o, 2, "oT")
            aop = ps_m([P, DM], "aoP")
            mm(aop, ones1, cs["bo"], start=True, stop=False)
            for fc in range(2):
                mm(aop, oT[:, fc, :], cs["Wo"][:, fc, :], start=False,
                   stop=(fc == 1))
            h1 = layer_norm(aop, h_nm_last, cs["g1"], cs["be1"], "1")
            h1T = transpose128(h1, 2, "h1T")
            zT = work.tile([P, 16, P], f16, name="zT", bufs=1)
            for half in range(2):
                zp = ps_big("SP")
                for s8 in range(8):
                    ffc = half * 8 + s8
                    for fc in range(2):
                        mm(zp[:, s8, :], cs["W1"][:, fc, ffc * P:(ffc + 1) * P],
                           h1T[:, fc, :], start=(fc == 0), stop=(fc == 1))
                for s8 in range(8):
                    ffc = half * 8 + s8
                    if ffc % 2 == 0:
                        S.activation(out=zT[:, ffc, :], in_=zp[:, s8, :],
                                     func=AF.Relu,
                                     bias=cs["b1fc"][:, ffc:ffc + 1])
                    else:
                        V.tensor_scalar(out=zT[:, ffc, :], in0=zp[:, s8, :],
                                        scalar1=cs["b1fc"][:, ffc:ffc + 1],
                                        scalar2=0.0, op0=OP.add, op1=OP.max)
            y2p = ps_m([P, DM], "y2P")
            mm(y2p, ones1, cs["b2f"], start=True, stop=False)
            for ffc in range(16):
                mm(y2p, zT[:, ffc, :], cs["W2"][:, ffc, :],
                   start=False, stop=(ffc == 15))
            xw = ps_m([P, F - 2], "xwP")
            mm(xw, ones1, cs["c1r"], start=True, stop=False)
            for fc in range(2):
                mm(xw, h1T[:, fc, :], cs["Wd"][:, fc, :],
                   start=False, stop=False)
            for ffc in range(16):
                mm(xw, zT[:, ffc, :], cs["W2d"][:, ffc, :],
                   start=False, stop=(ffc == 15))
            t1 = work.tile([P, DM], f32, name="ln_t1_2")
            V.tensor_tensor(out=t1, in0=y2p, in1=h1, op=OP.add)
            st = work.tile([P, 6], f32, name="ln_st_2")
            V.bn_stats(out=st, in_=t1)
            mv = work.tile([P, 2], f32, name="ln_mv_2")
            V.bn_aggr(out=mv, in_=st)
            vv = work.tile([P, 1], f32, name="ln_vv_2")
            V.tensor_scalar_add(out=vv, in0=mv[:, 1:2], scalar1=1e-5)
            rs = work.tile([P, 1], f32, name="ln_rs_2")
            rsi = rs.bitcast(mybir.dt.int32)
            V.tensor_scalar(out=rsi, in0=vv.bitcast(mybir.dt.int32),
                            scalar1=1, scalar2=None, op0=OP.arith_shift_right)
            V.tensor_scalar(out=rsi, in0=rsi, scalar1=-1, scalar2=0x5F3759DF,
                            op0=OP.mult, op1=OP.add)
            t_n = work.tile([P, 1], f32, name="ln_nt_2")
            V.tensor_tensor(out=t_n, in0=rs, in1=rs, op=OP.mult)
            V.tensor_tensor(out=t_n, in0=t_n, in1=vv, op=OP.mult)
            V.tensor_scalar(out=t_n, in0=t_n, scalar1=-0.5, scalar2=1.5,
                            op0=OP.mult, op1=OP.add)
            V.tensor_tensor(out=rs, in0=rs, in1=t_n, op=OP.mult)
            xb = work.tile([P, F - 2], f32, name="xb")
            V.scalar_tensor_tensor(out=xb, in0=cs["nsB1b"], scalar=mv[:, 0:1],
                                   in1=xw, op0=OP.mult, op1=OP.add)
            # Critical-path copy elision: write the f16 next-step input
            # directly into the persistent xaug state; the f32 copy for the
            # output DMA follows off-path.
            V.scalar_tensor_tensor(out=xaug[:, 0:F - 2], in0=xb, scalar=rs,
                                   in1=cs["bdB1b"], op0=OP.mult, op1=OP.add)
            xn32 = work.tile([P, F - 2], f32, name="xn32")
            V.scalar_tensor_tensor(out=xn32, in0=xb, scalar=rs,
                                   in1=cs["bdB1b"], op0=OP.mult, op1=OP.add)
            return xn32

        # ------------------------------------------------------------------
        # Phase K: known steps with ONE batched AllGather
        # ------------------------------------------------------------------
        def tconv1_stage(midT, midT8, m8, idx, mask_v=False, early_root=False,
                         split=False):
            if split:
                # Two collectives: midT lands first (gates scores); the mid
                # payload rides behind and lands under the score phase.
                bink = dram.tile([P, 2 * P], f8, name="bink1")
                boutk = dram.tile([NC_ * P, 2 * P], f8, name="bout1",
                                  addr_space="Shared")
                bink2 = dram.tile([P, DM], f8, name="bink2")
                bout2 = dram.tile([NC_ * P, DM], f8, name="bout2",
                                  addr_space="Shared")
                nc.sync.dma_start(
                    out=bink.rearrange("p (c j) -> p c j", j=P), in_=midT8)
                G.dma_start(out=bink2, in_=m8)
                nc.gpsimd.collective_compute(
                    "AllGather", OP.bypass, replica_groups=RG,
                    ins=[bink[:]], outs=[boutk[:]])
                nc.gpsimd.collective_compute(
                    "AllGather", OP.bypass, replica_groups=RG,
                    ins=[bink2[:]], outs=[bout2[:]])
            else:
                bink = dram.tile([P, 2 * P + DM], f8, name="bink")
                boutk = dram.tile([NC_ * P, 2 * P + DM], f8, name="boutk",
                                  addr_space="Shared")
                nc.sync.dma_start(
                    out=bink[:, 0:2 * P].rearrange("p (c j) -> p c j", j=P),
                    in_=midT8)
                G.dma_start(out=bink[:, 2 * P:2 * P + DM], in_=m8)
                nc.gpsimd.collective_compute(
                    "AllGather", OP.bypass, replica_groups=RG,
                    ins=[bink[:]], outs=[boutk[:]])
            # Local work that overlaps the gather:
            M1sb = work.tile([P, 2, H, P], f8, name="M1sb", bufs=4)
            abt = cs["AhatBt"]
            for cb in range(2):
                mp = ps_m([P, H, P], "m1P")
                for h in range(H):
                    sl = slice(cb * P, (cb + 1) * P)
                    mm(mp[:, h, :], cs["Ahat18"][:, :, h, sl], midT8,
                       start=True, stop=True, perf_mode=DR)
                abv = abt[:, cb, :]
                ab_hd = bass.AP(tensor=abv.tensor, offset=abv.offset,
                                ap=[list(abv.ap[0]), [1, H], [0, P]])
                V.tensor_tensor(out=M1sb[:, cb, :, :], in0=mp, in1=ab_hd,
                                op=OP.add)
            brpe = cs["brpeC"][0:1, idx * DM:(idx + 1) * DM]
            rootP_sb = None
            if early_root:
                rp_t = psS.tile([P, H, P], f32, name="rootP", tag="Scc")
                rp = rp_t[:, 0:2, :].rearrange("p a b -> p (a b)")
                mm(rp, ones1, brpe, start=True, stop=False)
                for fc in range(2):
                    mm(rp, midT[:, fc, :], cs["Wr1"][:, fc, :],
                       start=False, stop=(fc == 1))
                rootP_sb = work.tile([P, DM], f32, name="rootPsb")
                S.copy(out=rootP_sb, in_=rp)
            gk = boutk.rearrange("(r p) cj -> r p cj", r=NC_)
            mT_all = big.tile([P, 2, 8, P], f8, name="mT_all", tag="kst", bufs=3)
            m_all = big.tile([P, 8, 272], f8, name="m_all", tag="vall", bufs=2)
            # In gen (split) the S/V queues are idle during the gather gap and
            # nothing behind these loads on S/V is needed before coll-1 lands,
            # so fan the midT loads over 4 queues: any score-chunk order the
            # scheduler picks then has its region resident ~immediately. In
            # the pipelined known phase S/V carry the previous step's attn
            # work, so only sync/G may block on the collective there.
            mT_eng = ((nc.sync, S, G) if split else (nc.sync, G))
            ne = len(mT_eng)
            for r in range(NC_):
                mT_eng[r % ne].dma_start(out=mT_all[:, :, r, :],
                                         in_=gk[r][:, 0:2 * P].rearrange(
                                             "p (c j) -> p c j", j=P))
            gm = (bout2 if split else boutk).rearrange(
                "(r p) cj -> r p cj", r=NC_)
            moff = 0 if split else 2 * P
            for r in range(NC_):
                ke = nc.sync if r < 4 else G
                ke.dma_start(out=m_all[:, r, 0:DM],
                             in_=gm[r][:, moff:moff + DM])
            V.memset(m_all[:, :, DM:DM + 1], 1.0)
            return (midT, mT_all, m_all, M1sb, brpe, idx, mask_v, rootP_sb)

        def tconv1_finish(midT, mT_all, m_all, M1sb, brpe, idx, mask_v=False,
                          rootP_sb=None):
            return tconv1_attn(midT, mT_all, m_all, M1sb, brpe, idx, mask_v,
                               rootP_sb)

        hT_last = None
        h_nm_last = None
        pending = []
        for i in range(N_KNOWN):
            kxTa = work.tile([AUG, N], f16, name="kxTa", bufs=4)
            nc.sync.dma_start(out=kxTa, in_=d["kxTaug_all"][i])
            kxa = work.tile([P, 8, 32], f8, name="kxa", bufs=4)
            nc.sync.dma_start(out=kxa, in_=d["kxaug_all"][i])
            kxTl = work.tile([AUG, P], f16, name="kxTl", bufs=4)
            nc.sync.dma_start(out=kxTl, in_=d["kxTaug_loc"][i])
            midT, midT8, m8 = tconv0(
                lambda cc, _t=kxTa: _t[:, cc * P:(cc + 1) * P],
                kxa, kxTl, mask_v=True)
            pending.append(tconv1_stage(midT, midT8, m8, i, mask_v=True))
            if len(pending) > 2:
                st = pending.pop(0)
                h_nm_last, hT_last = tconv1_finish(*st)
                cache_update(st[5], hT_last)
        for st in pending:
            h_nm_last, hT_last = tconv1_finish(*st)
            cache_update(st[5], hT_last)

        # ------------------------------------------------------------------
        # Phase G: autoregressive generation
        # ------------------------------------------------------------------
        for t in range(K, K + N_GEN):
            xn32 = enc(t, h_nm_last)
            if t == K:
                llb = d["latlon32"]
                ll10 = bass.AP(tensor=llb.tensor, offset=llb.offset,
                               ap=[[0, N_GEN]] + [list(a) for a in llb.ap])
                nc.scalar.dma_start(out=out_d[:, :, 0:2], in_=ll10)
            nc.scalar.dma_start(out=out_d[t - K, :, 2:F], in_=xn32)
            if t == K + N_GEN - 1 or t == NG - 1:
                break
            tp = ps_m([F - 2, P], "ptp", dtype=f16)
            mm(tp, xaug[:, 0:F - 2], idn, start=True, stop=True,
               is_transpose=True)
            V.tensor_copy(out=xTaug[0:F - 2, :], in_=tp)
            gin = dram.tile([1, 2 * AUG * P], f16, name="g_in")
            gout = dram.tile([NC_, 2 * AUG * P], f16, name="g_out",
                             addr_space="Shared")
            nc.sync.dma_start(
                out=gin[0, 0:AUG * P].rearrange("(p j) -> p j", p=AUG), in_=xTaug)
            G.dma_start(
                out=gin[0, AUG * P:2 * AUG * P].rearrange("(p j) -> p j", p=P),
                in_=xaug)
            nc.gpsimd.collective_compute(
                "AllGather", OP.bypass, replica_groups=RG,
                ins=[gin[:]], outs=[gout[:]])
            xTa_all = work.tile([AUG, 8, P], f16, name="xTa_all")
            nc.sync.dma_start(
                out=xTa_all,
                in_=gout[:, 0:AUG * P].rearrange("r (p j) -> p r j", p=AUG))
            xa_all = work.tile([P, 8, AUG], f16, name="xa_all")
            G.dma_start(
                out=xa_all,
                in_=gout[:, AUG * P:2 * AUG * P].rearrange("r (p j) -> p r j", p=P))
            V.tensor_copy(out=xa8_st[:, :, 0:AUG], in_=xa_all)
            midT, midT8, m8 = tconv0(lambda cc, _t=xTa_all: _t[:, cc, :],
                                     xa8_st, xTaug, mask_v=True)
            st = tconv1_stage(midT, midT8, m8, t, mask_v=True, early_root=True,
                              split=True)
            h_nm_last, hT_last = tconv1_finish(*st)
            cache_update(t, hT_last)

    nc.finalize()
    return nc


# ----------------------------------------------------------------------------
# Host-side preprocessing
# ----------------------------------------------------------------------------
def prep_in_maps(inputs):
    import ml_dtypes
    bf16np = ml_dtypes.bfloat16
    f8np = ml_dtypes.float8_e4m3
    f32 = np.float32
    f16 = np.float16
    g = {k: np.asarray(v) for k, v in inputs.items()}
    kx = g["known_x"].astype(f32)                       # [10, 1024, 10]
    ei = g["edge_index"].astype(np.int64)

    Cnt = np.zeros((N, N), f32)
    np.add.at(Cnt, (ei[0], ei[1]), 1.0)
    LT = np.where(Cnt > 0, np.log(np.maximum(Cnt, 1.0)), _NEG).astype(f32)

    isd = f32(1.0 / np.sqrt(D))
    PERM = [2, 3, 4, 5, 6, 7, 8, 9, 0, 1, 10]
    Wq0a = (np.vstack([g["Wq0"], g["bq0"][None]]).astype(f32) * isd)[PERM]
    Wk0a = np.vstack([g["Wk0"], g["bk0"][None]]).astype(f32)[PERM]
    A = np.stack([(Wk0a[:, h * D:(h + 1) * D] @ Wq0a[:, h * D:(h + 1) * D].T)
                  for h in range(H)])                                # [4, 11, 11]
    AT = A.transpose(0, 2, 1).transpose(1, 0, 2).copy()              # [11, 4, 11]

    kxaug = np.concatenate([kx, np.ones((K, N, 1), f32)], axis=2)[:, :, PERM]
    kxTaug = kxaug.transpose(0, 2, 1).copy()                         # [10, 11, 1024]

    ide = f32(1.0 / np.sqrt(DH))
    Wqkv, bqkv = g["Wqkv"].astype(f32), g["bqkv"].astype(f32)

    def w2t(w, nch):
        m = w.shape[1]
        return np.ascontiguousarray(
            np.asarray(w, f32).reshape(nch, P, m).transpose(1, 0, 2))

    Wv0a = np.vstack([g["Wv0"], g["bv0"][None]]).astype(f32)[PERM]   # [11, 1024]
    Wv0az = np.zeros((AUG, H, D + 1), f32)
    for h in range(H):
        Wv0az[:, h, 0:D] = Wv0a[:, h * D:(h + 1) * D]
    Wv0az[10, :, D] = 1.0      # e10 column (ones-row index under PERM) -> z

    # Layer-1 bilinear score form: s[dst,src] = u_dst @ Ahat_h @ mid_src^T
    # with u_dst = [mid_dst, 1]; per-dst-constant terms (q.bk) cancel in the
    # softmax over src, so the k-bias column is dropped.
    Wq1a = np.vstack([np.asarray(g["Wq1"], f32),
                      np.asarray(g["bq1"], f32)[None]]) * isd    # [257, 1024]
    Wk1f = np.asarray(g["Wk1"], f32)                             # [256, 1024]
    Ahat = np.stack([Wq1a[:, h * D:(h + 1) * D]
                     @ Wk1f[:, h * D:(h + 1) * D].T
                     for h in range(H)])                         # [H, 257, 256]
    Ahat1 = np.ascontiguousarray(
        Ahat[:, 0:256, :].reshape(H, 2, P, DM).transpose(2, 1, 0, 3))
    # AhatBt[c', cb, h] = Ahat_h[256, cb*128 + c'] (bq-row, added post-matmul)
    AhatBt = np.ascontiguousarray(
        Ahat[:, 256, :].reshape(H, 2, P).transpose(2, 1, 0))

    Wdp = np.asarray(g["g2"], f32)[:, None] * np.asarray(g["Wd"], f32)
    W2d8 = np.asarray(g["W2"], f32) @ Wdp                      # [2048, 8]
    c1v = np.asarray(g["b2f"], f32) @ Wdp                      # [8]
    negSv = -Wdp.sum(axis=0)                                   # [8]
    c2v = (np.asarray(g["bd"], f32)
           + np.asarray(g["be2"], f32) @ np.asarray(g["Wd"], f32))

    common = {
        "AT": AT.astype(f16),
        "Wv0az": Wv0az.astype(f16),
        "Wr0a": np.vstack([g["Wr0"], g["br0"][None]])[PERM].astype(f16),
        "Ahat1": Ahat1.astype(f16),
        "Ahat18": Ahat1.astype(f8np),
        "AhatBt": AhatBt.astype(f16),
        "Wv1": w2t(g["Wv1"], 2).astype(f16),
        "Wr1": w2t(g["Wr1"], 2).astype(f16),
        "brpe1": (np.asarray(g["br1"], f32)[None]
                  + np.asarray(g["bv1"], f32).reshape(H, D).mean(axis=0)[None]
                  + np.asarray(g["pe"], f32))[:, None, :].astype(f16),
        "Wqe": w2t(Wqkv[:, 0:DM] * ide, 2).astype(f16),
        "Wke": w2t(Wqkv[:, DM:2 * DM], 2).astype(f16),
        "Wve": w2t(Wqkv[:, 2 * DM:], 2).astype(f16),
        "bqe": (bqkv[0:DM] * ide)[None].astype(f16),
        "bke": bqkv[DM:2 * DM][None].astype(f16),
        "bve": bqkv[2 * DM:][None].astype(f16),
        "Wo": w2t(g["Wo"], 2).astype(f16),
        "bo": np.asarray(g["bo"], f16)[None],
        "W1": w2t(g["W1"], 2).astype(f16),
        "b1f": np.asarray(g["b1f"], f16)[None],
        "b1fc": np.ascontiguousarray(
            np.asarray(g["b1f"], f32).reshape(16, P).T),
        "W2": w2t(g["W2"], 16).astype(f16),
        "b2f": np.asarray(g["b2f"], f16)[None],
        "Wd": w2t(Wdp, 2).astype(f16),
        "W2d": w2t(W2d8, 16).astype(f16),
        "c1r": c1v[None].astype(f16),
        "nsB1": negSv[None].astype(f32),
        "bdB1": c2v[None].astype(f32),
        "bd": np.asarray(g["bd"], f16)[None],
        "g1": np.asarray(g["g1"], f32)[None],
        "be1": np.asarray(g["be1"], f32)[None],
        "g2": np.asarray(g["g2"], f32)[None],
        "be2": np.asarray(g["be2"], f32)[None],
        "idn": np.eye(P, dtype=f16),
        "ones1": np.ones((1, P), f16),
        "kxTaug_all": kxTaug.astype(f16),
        "kxaug_all": np.ascontiguousarray(
            np.concatenate([kxaug, np.zeros((K, N, 32 - AUG), f32)], axis=2)
            .reshape(K, 8, P, 32).transpose(0, 2, 1, 3)).astype(f8np),
    }
    in_maps = []
    for c in range(NC_):
        sl = slice(P * c, P * (c + 1))
        m = dict(common)
        m["lt"] = np.ascontiguousarray(
            LT[:, sl].reshape(8, P, P).transpose(1, 0, 2)).astype(f16)
        m["cnt"] = np.ascontiguousarray(
            Cnt[:, sl].reshape(8, P, P).transpose(1, 0, 2)).astype(f16)
        m["latlon32"] = np.ascontiguousarray(kx[K - 1, sl, 0:2]).astype(f32)
        ll = kx[K - 1, sl, 0:2].astype(f32)
        xti = np.zeros((AUG, P), f32); xti[8:10] = ll.T; xti[10] = 1.0
        m["xTaug_init"] = xti.astype(f16)
        xai = np.zeros((P, AUG), f32); xai[:, 8:10] = ll; xai[:, 10] = 1.0
        m["xaug_init"] = xai.astype(f16)
        m["kxTaug_loc"] = np.ascontiguousarray(kxTaug[:, :, sl]).astype(f16)
        in_maps.append(m)
    return in_maps


_CACHED = {}


def run(inputs, trace=False, trace_kwargs=None):
    from concourse import bass_utils
    if "nc" not in _CACHED:
        _CACHED["nc"] = build_bass()
    in_maps = prep_in_maps(inputs)
    res = bass_utils.run_bass_kernel_spmd(
        _CACHED["nc"], in_maps, core_ids=list(range(NC_)), trace=trace,
        **(trace_kwargs or {}))
    out = np.concatenate([res.results[c]["out"] for c in range(NC_)], axis=1)
    return out.astype(np.float32), res


def kernel(**inputs):
    out, _ = run(inputs, trace=False)
    return out



# revision 58
# speedup vs baseline: 1.0079x; 1.0079x over previous
"""Trainium2 Bass kernel for nn_AdaptiveConditionedGraphTransformer.

Strategy (8 NeuronCores, data-parallel over nodes, 128 nodes/core):
- Graph edge-attention (PyG TransformerConv) computed DENSELY per core as
  [src=1024, dst=128] score matrices on TensorE; duplicate edges + masking
  handled exactly via a host-precomputed ln(count) additive mask injected
  into PSUM with an identity matmul inside the score accumulation group.
- Layer-0 tconv (fin=10) uses a rank-11 factorization: S = x_aug A x_aug^T
  with A precomputed on host; all 4 heads' score matmuls fused into one
  N=512 instruction per src chunk. Only a 5.5KB AllGather of x per step.
- Layer-1 tconv scores use the same bilinear trick at rank 257:
  S_h = [mid,1] @ (Wq_aug_h Wk_h^T/16) @ mid_all^T, so no Q/K projections
  exist at all. ONE fp8 collective per step gathers mid in both layouts
  (midT for scores + node-major for aggregation, 64KB in); in the gen
  phase it is split in two so the node-major half lands under the score
  phase. No V gather: the attention aggregates the 256-dim mid directly
  (Pm_h = sum_src P_h mid_src), then projects Pm_h through Wv1_h after
  the softmax-weighted sum (transpose + per-head matmul); bv1's head-mean
  folds into the br1+pe constant since softmax weights sum to 1. M1 =
  Ahat^T @ [midT;1] is computed locally during the gather.
- Softmax denominators ride along in the aggregation matmuls (ones column
  appended to gathered mid / e10 column appended to Wv0aug) -- no separate
  z matmuls.
- Temporal transformer layer: exact KV-cache, last-query-only; q/k/v cache
  projections fused into one PE burst after tconv1; all-head score and
  output reductions as single strided multi-dim DVE ops; softmax 1/z
  applied on the 64-wide head outputs.
- All biases folded into broadcast-tile adds on the PSUM->SBUF copies or
  per-partition scalar ops -- no ones-row bias matmuls. FFN relu+bias and
  half the PSUM->SBUF staging copies run on ScalarE (relu/copy share the
  exp activation table set, so no table switches). Per-chunk exp staging
  tiles are allocated inside the chunk loops so the EXP(Scalar) ->
  mask-mult(DVE) chains double-buffer instead of serializing.
- LayerNorm rsqrt via bit-trick + one Newton step on DVE; ScalarE runs only
  Exp (single activation table set, no ~2.7us table switches).
- Matmul inputs fp16 (fp32 PSUM); fp8 e4m3 only on softmax/mean-protected
  paths (gathered midT + M1 for scores, layer-1 probs, gathered V), enabling
  DoubleRow (2 K-chunks/instruction) on the layer-1 score and aggregation
  matmuls; everything else f16 to hold rel_err ~4.5e-3.

kernel(**inputs) takes FULL inputs, shards internally, returns FULL output.
"""
import os
import sys

import numpy as np

sys.path.insert(0, "/opt/trn_rl_repo")

N, E, F = 1024, 16384, 10
DM, H = 256, 4
D = 256
NG, K = 20, 10
FF = 2048
DH = DM // H
NC_ = 8
P = N // NC_          # 128 nodes per core
AUG = F + 1           # 11
TCACHE = NG - 1       # 19 cache slots

N_KNOWN = int(os.environ.get("GT_KNOWN", "10"))
N_GEN = int(os.environ.get("GT_GEN", "10"))

_NEG = -30000.0


# ----------------------------------------------------------------------------
# Device program
# ----------------------------------------------------------------------------
def build_bass():
    import concourse.bass as bass
    import concourse.tile as tile
    from concourse import bacc, mybir

    f16 = mybir.dt.float16
    f32 = mybir.dt.float32
    f8 = mybir.dt.float8e4
    bf16 = mybir.dt.bfloat16
    AF = mybir.ActivationFunctionType
    OP = mybir.AluOpType
    AX = mybir.AxisListType

    nc = bacc.Bacc("TRN2", target_bir_lowering=False, debug=False, num_devices=NC_)
    RG = [list(range(NC_))]

    def din(name, shape, dtype):
        return nc.dram_tensor(name, list(shape), dtype, kind="ExternalInput").ap()

    d = {}
    d["lt"] = din("lt", [P, 8, P], f16)
    d["cnt"] = din("cnt", [P, 8, P], f16)
    d["latlon32"] = din("latlon32", [P, 2], f32)
    d["AT"] = din("AT", [AUG, H, AUG], f16)
    d["Ahat18"] = din("Ahat18", [P, 2, H, DM], f8)
    d["Wv0az"] = din("Wv0az", [AUG, H, D + 1], f16)
    d["Wr0a"] = din("Wr0a", [AUG, DM], f16)
    d["kxTaug_all"] = din("kxTaug_all", [K, AUG, N], f16)
    d["kxaug_all"] = din("kxaug_all", [K, P, 8, 32], f8)
    d["kxTaug_loc"] = din("kxTaug_loc", [K, AUG, P], f16)
    d["Ahat1"] = din("Ahat1", [P, 2, H, DM], f16)
    d["AhatBt"] = din("AhatBt", [P, 2, H], f16)
    d["Wv1"] = din("Wv1", [P, 2, H * D], f16)
    d["Wr1"] = din("Wr1", [P, 2, DM], f16)
    d["brpe1"] = din("brpe1", [NG, 1, DM], f16)
    d["Wqe"] = din("Wqe", [P, 2, DM], f16)
    d["Wke"] = din("Wke", [P, 2, DM], f16)
    d["Wve"] = din("Wve", [P, 2, DM], f16)
    d["Wo"] = din("Wo", [P, 2, DM], f16)
    d["bqe"] = din("bqe", [1, DM], f16)
    d["bke"] = din("bke", [1, DM], f16)
    d["bve"] = din("bve", [1, DM], f16)
    d["bo"] = din("bo", [1, DM], f16)
    d["W1"] = din("W1", [P, 2, FF], f16)
    d["b1f"] = din("b1f", [1, FF], f16)
    d["b1fc"] = din("b1fc", [P, 16], f32)
    d["W2"] = din("W2", [P, 16, DM], f16)
    d["b2f"] = din("b2f", [1, DM], f16)
    d["Wd"] = din("Wd", [P, 2, F - 2], f16)
    d["W2d"] = din("W2d", [P, 16, F - 2], f16)
    d["c1r"] = din("c1r", [1, F - 2], f16)
    d["nsB1"] = din("nsB1", [1, F - 2], f32)
    d["bdB1"] = din("bdB1", [1, F - 2], f32)
    d["bd"] = din("bd", [1, F - 2], f16)
    d["g1"] = din("g1", [1, DM], f32)
    d["be1"] = din("be1", [1, DM], f32)
    d["g2"] = din("g2", [1, DM], f32)
    d["be2"] = din("be2", [1, DM], f32)
    d["xTaug_init"] = din("xTaug_init", [AUG, P], f16)
    d["xaug_init"] = din("xaug_init", [P, AUG], f16)
    d["idn"] = din("idn", [P, P], f16)
    d["ones1"] = din("ones1", [1, P], f16)

    out_d = nc.dram_tensor("out", [N_GEN, P, F], f32, kind="ExternalOutput").ap()

    mm = nc.tensor.matmul
    DR = mybir.MatmulPerfMode.DoubleRow
    V = nc.vector
    S = nc.scalar
    G = nc.gpsimd

    def bc(ap, ins_size):
        """Insert a stride-0 dim before the last free dim: [p, n] -> [p, m, n]."""
        return bass.AP(tensor=ap.tensor, offset=ap.offset,
                       ap=[list(ap.ap[0]), [0, ins_size], list(ap.ap[1])])

    from contextlib import ExitStack
    with tile.TileContext(nc) as tc, ExitStack() as ctx:
        consts = ctx.enter_context(tc.tile_pool(name="consts", bufs=1))
        state = ctx.enter_context(tc.tile_pool(name="state", bufs=1))
        work = ctx.enter_context(tc.tile_pool(name="work", bufs=2))
        big = ctx.enter_context(tc.tile_pool(name="big", bufs=1))
        psA = ctx.enter_context(tc.tile_pool(name="psA", bufs=2, space="PSUM"))
        psS = ctx.enter_context(tc.tile_pool(name="psS", bufs=2, space="PSUM"))
        psM = ctx.enter_context(tc.tile_pool(name="psM", bufs=2, space="PSUM"))
        dram = ctx.enter_context(tc.tile_pool(name="dram", bufs=2, space="DRAM"))

        def ps_big(name, shape=(P, 8, P)):
            return psA.tile(list(shape), f32, name=name, tag="SP")

        def ps_m(shape, name, dtype=None):
            return psM.tile(list(shape), dtype or f32, name=name, tag="mP")

        cs = {}
        _skip = ("nsB1", "bdB1", "g1", "be1", "g2", "be2", "kxTaug_all", "kxaug_all",
                 "kxTaug_loc", "xTaug_init", "xaug_init", "latlon32", "brpe1")
        _early = ("AT", "idn", "ones1", "lt", "cnt", "Wv0az", "Wr0a", "Ahat1",
                  "AhatBt", "Wv1", "Wr1")
        _order = [n for n in _early if n in d] + \
                 [n for n in d if n not in _early and n not in _skip]
        _eng = [nc.sync, nc.scalar, nc.sync, nc.scalar]
        for _i, name in enumerate(_order):
            ap = d[name]
            t = consts.tile(list(ap.shape), ap.dtype, name=f"c_{name}")
            _eng[_i % 4].dma_start(out=t, in_=ap)
            cs[name] = t
        for name in ("g1", "be1", "g2", "be2"):
            t = consts.tile([P, DM], f32, name=f"c_{name}")
            nc.sync.dma_start(out=t, in_=d[name].to_broadcast((P, DM)))
            cs[name] = t
        for name in ("nsB1", "bdB1"):
            t = consts.tile([P, F - 2], f32, name=f"c_{name}b")
            nc.scalar.dma_start(out=t, in_=d[name].to_broadcast((P, F - 2)))
            cs[name + "b"] = t
        for nm, src_nm, w in (("bkeB", "bke", DM), ("bveB", "bve", DM),
                              ("bqeB", "bqe", DM)):
            t = consts.tile([P, w], f16, name=f"c_{nm}")
            nc.sync.dma_start(out=t, in_=d[src_nm].to_broadcast((P, w)))
            cs[nm] = t
        t = consts.tile([P, 16], f32, name="c_b1fc")
        nc.sync.dma_start(out=t, in_=d["b1fc"])
        cs["b1fc"] = t
        t = consts.tile([1, NG * DM], f16, name="c_brpeC")
        nc.scalar.dma_start(out=t, in_=d["brpe1"].rearrange("g o m -> o (g m)"))
        cs["brpeC"] = t
        idn = cs["idn"]; ones1 = cs["ones1"]
        lt = cs["lt"]; cnt = cs["cnt"]

        Kc = state.tile([P, TCACHE, DM], f16, name="Kc")
        Vc2 = state.tile([P, H, DH, TCACHE], f16, name="Vc2")
        xTaug = state.tile([AUG, P], f16, name="xTaug")
        xaug = state.tile([P, AUG], f16, name="xaug")
        xa8_st = state.tile([P, 8, 32], f8, name="xa8_st")
        V.memset(xa8_st[:, :, AUG:32], 0.0)
        nc.sync.dma_start(out=xTaug, in_=d["xTaug_init"])
        nc.sync.dma_start(out=xaug, in_=d["xaug_init"])

        # ------------------------------------------------------------------
        def transpose128(src_ap, n_chunks, name, also_f8=False):
            dst = work.tile([P, n_chunks, P], f16, name=name,
                            bufs=(5 if name == "midT" else None))
            dst8 = (work.tile([P, n_chunks, P], f8, name=name + "8", bufs=4)
                    if also_f8 else None)
            for fc in range(n_chunks):
                pt = ps_m([P, P], "ptp", dtype=f16)
                mm(pt, src_ap[:, fc * P:(fc + 1) * P], idn, start=True, stop=True,
                   is_transpose=True)
                V.tensor_copy(out=dst[:, fc, :], in_=pt)
                if also_f8:
                    V.tensor_copy(out=dst8[:, fc, :], in_=pt)
            if also_f8:
                return dst, dst8
            return dst

        def combine(agg01, agg23, rootP, name):
            """sum_h agg_h/(4 z_h) + rootP; z_h rides in agg col 256.
            Head-pair 0/1 is scaled with its own zi tiles so that work can
            start as soon as agg01 stops, overlapping head-2/3 matmuls.
            All zi ops are whole-tile, never in-place (slice-in-place
            reciprocals raced and produced NaN)."""
            zi01 = work.tile([P, 2], f32, name=f"zi01_{name}")
            V.tensor_scalar(out=zi01,
                            in0=agg01[:, :, 256:257].rearrange("p h x -> p (h x)"),
                            scalar1=4.0, scalar2=4e-16, op0=OP.mult, op1=OP.add)
            zr01 = work.tile([P, 2], f32, name=f"zr01_{name}")
            V.reciprocal(out=zr01, in_=zi01)
            t0 = work.tile([P, DM], f32, name=f"cmb_{name}")
            V.tensor_scalar_mul(out=t0, in0=agg01[:, 0, 0:256],
                                scalar1=zr01[:, 0:1])
            V.scalar_tensor_tensor(out=t0, in0=agg01[:, 1, 0:256],
                                   scalar=zr01[:, 1:2], in1=t0,
                                   op0=OP.mult, op1=OP.add)
            zi23 = work.tile([P, 2], f32, name=f"zi23_{name}")
            V.tensor_scalar(out=zi23,
                            in0=agg23[:, :, 256:257].rearrange("p h x -> p (h x)"),
                            scalar1=4.0, scalar2=4e-16, op0=OP.mult, op1=OP.add)
            zr23 = work.tile([P, 2], f32, name=f"zr23_{name}")
            V.reciprocal(out=zr23, in_=zi23)
            V.scalar_tensor_tensor(out=t0, in0=agg23[:, 0, 0:256],
                                   scalar=zr23[:, 0:1], in1=t0,
                                   op0=OP.mult, op1=OP.add)
            V.scalar_tensor_tensor(out=t0, in0=agg23[:, 1, 0:256],
                                   scalar=zr23[:, 1:2], in1=t0,
                                   op0=OP.mult, op1=OP.add)
            out_nm = work.tile([P, DM], f16, name=f"nm_{name}")
            V.tensor_tensor(out=out_nm, in0=rootP, in1=t0, op=OP.add)
            return out_nm

        def tconv0(xTa_all_fn, xa8, xTa_loc, mask_v=False):
            """Rank-11 layer-0 tconv."""
            MpP = ps_m([AUG, H, P], "MpP")
            for h in range(H):
                mm(MpP[:, h, :], cs["AT"][:, h, :], xTa_loc, start=True, stop=True)
            Mp = work.tile([AUG, H, P], f16, name="Mp")
            V.tensor_copy(out=Mp, in_=MpP)
            rootP = ps_m([P, DM], "rootP")
            mm(rootP, xTa_loc, cs["Wr0a"], start=True, stop=True)
            PT = big.tile([P, 8, H, P], f8, name="PT0", tag="PT0", bufs=2)
            GTP = ps_m([32, H, P], "GTP32")
            assert mask_v
            # Chunk-paired scores: one Exp and one cnt-mask multiply cover two
            # src chunks, amortizing per-op PSUM-access/dispatch overhead. The
            # paired [P, 2, H, P] psum reuses the 4KB "SP" slots.
            for cp in range(4):
                sp2 = psA.tile([P, 2, H, P], f32, name="sp2", tag="SP")
                for j in range(2):
                    mm(sp2[:, j, :, :], xTa_all_fn(2 * cp + j), Mp,
                       start=True, stop=True)
                PTe0 = work.tile([P, 2, H, P], f16, name="PTe0", bufs=3)
                S.activation(out=PTe0, in_=sp2, func=AF.Exp)
                cv = cnt[:, 2 * cp:2 * cp + 2, :]
                cnt2 = bass.AP(tensor=cv.tensor, offset=cv.offset,
                               ap=[list(cv.ap[0]), list(cv.ap[1]), [0, H],
                                   list(cv.ap[2])])
                V.tensor_tensor(out=PT[:, 2 * cp:2 * cp + 2, :, :], in0=PTe0,
                                in1=cnt2, op=OP.mult)
            for cp in range(4):
                mm(GTP, xa8[:, 2 * cp:2 * cp + 2, :],
                   PT[:, 2 * cp:2 * cp + 2, :, :],
                   start=(cp == 0), stop=(cp == 3), perf_mode=DR)
            GT = work.tile([AUG, H, P], f16, name="GT")
            V.tensor_copy(out=GT, in_=GTP[0:AUG, :, :])
            agg01 = ps_big("agg01", (P, 2, 512))
            agg23 = ps_big("agg23", (P, 2, 512))
            for h in range(H):
                dst = (agg01, agg23)[h // 2][:, h % 2, 0:D + 1]
                mm(dst, GT[:, h, :], cs["Wv0az"][:, h, :], start=True, stop=True)
            mid = combine(agg01, agg23, rootP, "t0")
            m8 = work.tile([P, DM], f8, name="m8", bufs=4)
            S.copy(out=m8, in_=mid)
            midT, midT8 = transpose128(mid, 2, "midT", also_f8=True)
            return midT, midT8, m8

        def tconv1_attn(midT, mT_all, m_all, M1sb, brpe, idx, mask_v=False,
                        rootP_sb=None):
            """Dense edge attention from gathered midT/mid. Aggregates the
            256-dim mid per head (z rides col 256 via the m_all ones col),
            then projects through Wv1_h after the softmax-weighted sum.
            Returns (h_nm, hT)."""
            PT = big.tile([P, H, 8, P], f8, name="PT", tag="PT", bufs=2)
            assert mask_v
            # Per-chunk pipeline (pairing regressed here: the paired fp8-out
            # mask multiply at DVE 1x became the pacer; per-chunk keeps
            # mm/exp/mult balanced at ~0.8us each).
            for cc in range(8):
                sp = psS.tile([P, H, P], f32, name="Scc", tag="Scc")
                PTe = work.tile([P, H, P], f16, name="PTe", bufs=3)
                mm(sp, mT_all[:, :, cc, :], M1sb, start=True, stop=True,
                   perf_mode=DR)
                S.activation(out=PTe, in_=sp, func=AF.Exp)
                V.tensor_tensor(out=PT[:, :, cc, :], in0=PTe,
                                in1=bc(cnt[:, cc, :], H), op=OP.mult)
            agg01 = ps_big("agg01", (P, 2, 512))
            agg23 = ps_big("agg23", (P, 2, 512))
            for h in range(H):
                dst = (agg01, agg23)[h // 2][:, h % 2, 0:D + 1]
                for cp in range(4):
                    mm(dst, PT[:, h, 2 * cp:2 * cp + 2, :],
                       m_all[:, 2 * cp:2 * cp + 2, 0:D + 1],
                       start=(cp == 0), stop=(cp == 3), perf_mode=DR)
            # 1/(4 z_h) from the ones-column sums.
            zi = work.tile([P, H], f32, name="zi_t1")
            V.tensor_scalar(out=zi[:, 0:2],
                            in0=agg01[:, :, 256:257].rearrange("p h x -> p (h x)"),
                            scalar1=4.0, scalar2=4e-16, op0=OP.mult, op1=OP.add)
            V.tensor_scalar(out=zi[:, 2:4],
                            in0=agg23[:, :, 256:257].rearrange("p h x -> p (h x)"),
                            scalar1=4.0, scalar2=4e-16, op0=OP.mult, op1=OP.add)
            V.reciprocal(out=zi, in_=zi)
            # Pm [dst, f] -> SBUF f16, transpose to [f, dst], project by Wv1_h.
            Pmf = work.tile([P, H, 2, P], f16, name="Pmf")
            V.tensor_copy(out=Pmf[:, 0, :, :],
                          in_=agg01[:, 0, 0:256].rearrange("p (c j) -> p c j", j=P))
            S.copy(out=Pmf[:, 1, :, :],
                   in_=agg01[:, 1, 0:256].rearrange("p (c j) -> p c j", j=P))
            V.tensor_copy(out=Pmf[:, 2, :, :],
                          in_=agg23[:, 0, 0:256].rearrange("p (c j) -> p c j", j=P))
            S.copy(out=Pmf[:, 3, :, :],
                   in_=agg23[:, 1, 0:256].rearrange("p (c j) -> p c j", j=P))
            PmT = work.tile([P, H, 2, P], f16, name="PmT")
            for h in range(H):
                for fc in range(2):
                    pt = ps_m([P, P], "ptp", dtype=f16)
                    mm(pt, Pmf[:, h, fc, :], idn, start=True, stop=True,
                       is_transpose=True)
                    if (h * 2 + fc) % 2 == 0:
                        V.tensor_copy(out=PmT[:, h, fc, :], in_=pt)
                    else:
                        S.copy(out=PmT[:, h, fc, :], in_=pt)
            projP = psA.tile([P, H, DM], f32, name="projP", tag="SP")
            for h in range(H):
                for fc in range(2):
                    mm(projP[:, h, :], PmT[:, h, fc, :],
                       cs["Wv1"][:, fc, h * D:(h + 1) * D],
                       start=(fc == 0), stop=(fc == 1))
            if rootP_sb is None:
                rootP_t = psS.tile([P, H, P], f32, name="rootP", tag="Scc")
                rootP_sb = rootP_t[:, 0:2, :].rearrange("p a b -> p (a b)")
                mm(rootP_sb, ones1, brpe, start=True, stop=False)
                for fc in range(2):
                    mm(rootP_sb, midT[:, fc, :], cs["Wr1"][:, fc, :],
                       start=False, stop=(fc == 1))
            t0 = work.tile([P, DM], f32, name="cmb_t1")
            V.tensor_scalar_mul(out=t0, in0=projP[:, 0, :], scalar1=zi[:, 0:1])
            for h in range(1, H):
                V.scalar_tensor_tensor(out=t0, in0=projP[:, h, :],
                                       scalar=zi[:, h:h + 1], in1=t0,
                                       op0=OP.mult, op1=OP.add)
            h_nm = work.tile([P, DM], f16, name="nm_t1")
            V.tensor_tensor(out=h_nm, in0=rootP_sb, in1=t0, op=OP.add)
            hT = transpose128(h_nm, 2, "hT")
            return h_nm, hT

        qe_st = state.tile([P, DM], f16, name="qe_st")

        def cache_update(slot, hT):
            # k and q first: enc's score chain needs them; v only feeds the
            # later attention-out reduction.
            for W, b, which in ((cs["Wke"], "bkeB", "k"),
                                (cs["Wqe"], "bqeB", "q"),
                                (cs["Wve"], "bveB", "v")):
                pp_t = psS.tile([P, H, P], f32, name="cuP", tag="Scc")
                pp = pp_t[:, 0:2, :].rearrange("p a b -> p (a b)")
                for fc in range(2):
                    mm(pp, hT[:, fc, :], W[:, fc, :], start=(fc == 0),
                       stop=(fc == 1))
                bB = cs[b]
                if which == "k":
                    V.tensor_tensor(out=Kc[:, slot, :], in0=pp, in1=bB, op=OP.add)
                elif which == "v":
                    V.tensor_tensor(out=Vc2[:, :, :, slot],
                                    in0=pp.rearrange("p (h e) -> p h e", h=H),
                                    in1=bB.rearrange("p (h e) -> p h e", h=H),
                                    op=OP.add)
                else:
                    V.tensor_tensor(out=qe_st, in0=pp, in1=bB, op=OP.add)

        def layer_norm(x_f32_psum, resid_f16, g, be, name):
            t1 = work.tile([P, DM], f32, name=f"ln_t1_{name}")
            V.tensor_tensor(out=t1, in0=x_f32_psum, in1=resid_f16, op=OP.add)
            st = work.tile([P, 6], f32, name=f"ln_st_{name}")
            V.bn_stats(out=st, in_=t1)
            mv = work.tile([P, 2], f32, name=f"ln_mv_{name}")
            V.bn_aggr(out=mv, in_=st)
            vv = work.tile([P, 1], f32, name=f"ln_vv_{name}")
            V.tensor_scalar_add(out=vv, in0=mv[:, 1:2], scalar1=1e-5)
            rs = work.tile([P, 1], f32, name=f"ln_rs_{name}")
            rsi = rs.bitcast(mybir.dt.int32)
            V.tensor_scalar(out=rsi, in0=vv.bitcast(mybir.dt.int32),
                            scalar1=1, scalar2=None, op0=OP.arith_shift_right)
            V.tensor_scalar(out=rsi, in0=rsi, scalar1=-1, scalar2=0x5F3759DF,
                            op0=OP.mult, op1=OP.add)
            t_n = work.tile([P, 1], f32, name=f"ln_nt_{name}")
            for _ in range(2):
                V.tensor_tensor(out=t_n, in0=rs, in1=rs, op=OP.mult)
                V.tensor_tensor(out=t_n, in0=t_n, in1=vv, op=OP.mult)
                V.tensor_scalar(out=t_n, in0=t_n, scalar1=-0.5, scalar2=1.5,
                                op0=OP.mult, op1=OP.add)
                V.tensor_tensor(out=rs, in0=rs, in1=t_n, op=OP.mult)
                break
            V.tensor_scalar(out=t1, in0=t1, scalar1=mv[:, 0:1], scalar2=rs,
                            op0=OP.subtract, op1=OP.mult)
            V.tensor_tensor(out=t1, in0=t1, in1=g, op=OP.mult)
            o = work.tile([P, DM], f16, name=f"ln_o_{name}")
            V.tensor_tensor(out=o, in0=t1, in1=be, op=OP.add)
            return o

        def enc(t, h_nm_last):
            # Scores for all heads in two fused DVE ops: Kc viewed [p,h,t,e]
            # (strided) times q broadcast over t, then innermost reduce.
            sc = work.tile([P, H, TCACHE], f16, name="sc")
            tmp = work.tile([P, H, TCACHE, DH], f16, name="sctmp", tag="etmp")
            kcv = Kc[:, 0:t, :]
            kc_htE = bass.AP(tensor=kcv.tensor, offset=kcv.offset,
                             ap=[list(kcv.ap[0]), [DH, H], [DM, t], [1, DH]])
            qv = qe_st[:]
            q_htE = bass.AP(tensor=qv.tensor, offset=qv.offset,
                            ap=[list(qv.ap[0]), [DH, H], [0, t], [1, DH]])
            V.tensor_tensor(out=tmp[:, :, 0:t, :], in0=kc_htE, in1=q_htE,
                            op=OP.mult)
            with nc.allow_low_precision("f16 attn scores, |s|<~4"):
                V.tensor_tensor(out=tmp[:, :, 0:t, 0:DH // 2],
                                in0=tmp[:, :, 0:t, 0:DH // 2],
                                in1=tmp[:, :, 0:t, DH // 2:DH], op=OP.add)
                V.tensor_tensor(out=tmp[:, :, 0:t, 0:DH // 4],
                                in0=tmp[:, :, 0:t, 0:DH // 4],
                                in1=tmp[:, :, 0:t, DH // 4:DH // 2], op=OP.add)
                V.tensor_tensor(out=tmp[:, :, 0:t, 0:DH // 8],
                                in0=tmp[:, :, 0:t, 0:DH // 8],
                                in1=tmp[:, :, 0:t, DH // 8:DH // 4], op=OP.add)
                V.tensor_reduce(out=sc[:, :, 0:t],
                                in_=tmp[:, :, 0:t, 0:DH // 8],
                                axis=AX.X, op=OP.add)
            S.activation(out=sc[:, :, 0:t], in_=sc[:, :, 0:t], func=AF.Exp)
            z = work.tile([P, H], f32, name="ze")
            V.tensor_reduce(out=z, in_=sc[:, :, 0:t], axis=AX.X, op=OP.add)
            V.reciprocal(out=z, in_=z)
            o = work.tile([P, DM], f16, name="oe")
            ow = work.tile([P, H, DH], f32, name="ow")
            tmp2 = work.tile([P, H, DH, TCACHE], f16, name="otmp", tag="etmp")
            scv = sc[:]
            sc_hEt = bass.AP(tensor=scv.tensor, offset=scv.offset,
                             ap=[list(scv.ap[0]), [TCACHE, H], [0, DH], [1, t]])
            V.tensor_tensor(out=tmp2[:, :, :, 0:t], in0=Vc2[:, :, :, 0:t],
                            in1=sc_hEt, op=OP.mult)
            th = (t + 1) // 2
            th2 = (th + 1) // 2
            with nc.allow_low_precision("f16 attn out, probs sum to 1"):
                V.tensor_tensor(out=tmp2[:, :, :, 0:t // 2],
                                in0=tmp2[:, :, :, 0:t // 2],
                                in1=tmp2[:, :, :, th:th + t // 2], op=OP.add)
                V.tensor_tensor(out=tmp2[:, :, :, 0:th // 2],
                                in0=tmp2[:, :, :, 0:th // 2],
                                in1=tmp2[:, :, :, th2:th2 + th // 2], op=OP.add)
                V.tensor_reduce(out=ow, in_=tmp2[:, :, :, 0:th2], axis=AX.X,
                                op=OP.add)
            for h in range(H):
                V.tensor_scalar_mul(out=o[:, h * DH:(h + 1) * DH],
                                    in0=ow[:, h, :], scalar1=z[:, h:h + 1])
            oT = transpose128(o, 2, "oT")
            aop = ps_m([P, DM], "aoP")
            mm(aop, ones1, cs["bo"], start=True, stop=False)
            for fc in range(2):
                mm(aop, oT[:, fc, :], cs["Wo"][:, fc, :], start=False,
                   stop=(fc == 1))
            h1 = layer_norm(aop, h_nm_last, cs["g1"], cs["be1"], "1")
            h1T = transpose128(h1, 2, "h1T")
            zT = work.tile([P, 16, P], f16, name="zT", bufs=1)
            for half in range(2):
                zp = ps_big("SP")
                for s8 in range(8):
                    ffc = half * 8 + s8
                    for fc in range(2):
                        mm(zp[:, s8, :], cs["W1"][:, fc, ffc * P:(ffc + 1) * P],
                           h1T[:, fc, :], start=(fc == 0), stop=(fc == 1))
                for s8 in range(8):
                    ffc = half * 8 + s8
                    if ffc % 2 == 0:
                        S.activation(out=zT[:, ffc, :], in_=zp[:, s8, :],
                                     func=AF.Relu,
                                     bias=cs["b1fc"][:, ffc:ffc + 1])
                    else:
                        V.tensor_scalar(out=zT[:, ffc, :], in0=zp[:, s8, :],
                                        scalar1=cs["b1fc"][:, ffc:ffc + 1],
                                        scalar2=0.0, op0=OP.add, op1=OP.max)
            y2p = ps_m([P, DM], "y2P")
            mm(y2p, ones1, cs["b2f"], start=True, stop=False)
            for ffc in range(16):
                mm(y2p, zT[:, ffc, :], cs["W2"][:, ffc, :],
                   start=False, stop=(ffc == 15))
            xw = ps_m([P, F - 2], "xwP")
            mm(xw, ones1, cs["c1r"], start=True, stop=False)
            for fc in range(2):
                mm(xw, h1T[:, fc, :], cs["Wd"][:, fc, :],
                   start=False, stop=False)
            for ffc in range(16):
                mm(xw, zT[:, ffc, :], cs["W2d"][:, ffc, :],
                   start=False, stop=(ffc == 15))
            t1 = work.tile([P, DM], f32, name="ln_t1_2")
            V.tensor_tensor(out=t1, in0=y2p, in1=h1, op=OP.add)
            st = work.tile([P, 6], f32, name="ln_st_2")
            V.bn_stats(out=st, in_=t1)
            mv = work.tile([P, 2], f32, name="ln_mv_2")
            V.bn_aggr(out=mv, in_=st)
            vv = work.tile([P, 1], f32, name="ln_vv_2")
            V.tensor_scalar_add(out=vv, in0=mv[:, 1:2], scalar1=1e-5)
            rs = work.tile([P, 1], f32, name="ln_rs_2")
            rsi = rs.bitcast(mybir.dt.int32)
            V.tensor_scalar(out=rsi, in0=vv.bitcast(mybir.dt.int32),
                            scalar1=1, scalar2=None, op0=OP.arith_shift_right)
            V.tensor_scalar(out=rsi, in0=rsi, scalar1=-1, scalar2=0x5F3759DF,
                            op0=OP.mult, op1=OP.add)
            t_n = work.tile([P, 1], f32, name="ln_nt_2")
            V.tensor_tensor(out=t_n, in0=rs, in1=rs, op=OP.mult)
            V.tensor_tensor(out=t_n, in0=t_n, in1=vv, op=OP.mult)
            V.tensor_scalar(out=t_n, in0=t_n, scalar1=-0.5, scalar2=1.5,
                            op0=OP.mult, op1=OP.add)
            V.tensor_tensor(out=rs, in0=rs, in1=t_n, op=OP.mult)
            xb = work.tile([P, F - 2], f32, name="xb")
            V.scalar_tensor_tensor(out=xb, in0=cs["nsB1b"], scalar=mv[:, 0:1],
                                   in1=xw, op0=OP.mult, op1=OP.add)
            # Critical-path copy elision: write the f16 next-step input
            # directly into the persistent xaug state; the f32 copy for the
            # output DMA follows off-path.
            V.scalar_tensor_tensor(out=xaug[:, 0:F - 2], in0=xb, scalar=rs,
                                   in1=cs["bdB1b"], op0=OP.mult, op1=OP.add)
            xn32 = work.tile([P, F - 2], f32, name="xn32")
            V.scalar_tensor_tensor(out=xn32, in0=xb, scalar=rs,
                                   in1=cs["bdB1b"], op0=OP.mult, op1=OP.add)
            return xn32

        # ------------------------------------------------------------------
        # Phase K: known steps with ONE batched AllGather
        # ------------------------------------------------------------------
        def tconv1_stage(midT, midT8, m8, idx, mask_v=False, early_root=False,
                         split=False):
            if split:
                # Two collectives: midT lands first (gates scores); the mid
                # payload rides behind and lands under the score phase.
                bink = dram.tile([P, 2 * P], f8, name="bink1")
                boutk = dram.tile([NC_ * P, 2 * P], f8, name="bout1",
                                  addr_space="Shared")
                bink2 = dram.tile([P, DM], f8, name="bink2")
                bout2 = dram.tile([NC_ * P, DM], f8, name="bout2",
                                  addr_space="Shared")
                nc.sync.dma_start(
                    out=bink.rearrange("p (c j) -> p c j", j=P), in_=midT8)
                G.dma_start(out=bink2, in_=m8)
                nc.gpsimd.collective_compute(
                    "AllGather", OP.bypass, replica_groups=RG,
                    ins=[bink[:]], outs=[boutk[:]])
                nc.gpsimd.collective_compute(
                    "AllGather", OP.bypass, replica_groups=RG,
                    ins=[bink2[:]], outs=[bout2[:]])
            else:
                bink = dram.tile([P, 2 * P + DM], f8, name="bink")
                boutk = dram.tile([NC_ * P, 2 * P + DM], f8, name="boutk",
                                  addr_space="Shared")
                nc.sync.dma_start(
                    out=bink[:, 0:2 * P].rearrange("p (c j) -> p c j", j=P),
                    in_=midT8)
                G.dma_start(out=bink[:, 2 * P:2 * P + DM], in_=m8)
                nc.gpsimd.collective_compute(
                    "AllGather", OP.bypass, replica_groups=RG,
                    ins=[bink[:]], outs=[boutk[:]])
            # Local work that overlaps the gather:
            M1sb = work.tile([P, 2, H, P], f8, name="M1sb", bufs=4)
            abt = cs["AhatBt"]
            for cb in range(2):
                mp = ps_m([P, H, P], "m1P")
                for h in range(H):
                    sl = slice(cb * P, (cb + 1) * P)
                    mm(mp[:, h, :], cs["Ahat18"][:, :, h, sl], midT8,
                       start=True, stop=True, perf_mode=DR)
                abv = abt[:, cb, :]
                ab_hd = bass.AP(tensor=abv.tensor, offset=abv.offset,
                                ap=[list(abv.ap[0]), [1, H], [0, P]])
                V.tensor_tensor(out=M1sb[:, cb, :, :], in0=mp, in1=ab_hd,
                                op=OP.add)
            brpe = cs["brpeC"][0:1, idx * DM:(idx + 1) * DM]
            rootP_sb = None
            if early_root:
                rp_t = psS.tile([P, H, P], f32, name="rootP", tag="Scc")
                rp = rp_t[:, 0:2, :].rearrange("p a b -> p (a b)")
                mm(rp, ones1, brpe, start=True, stop=False)
                for fc in range(2):
                    mm(rp, midT[:, fc, :], cs["Wr1"][:, fc, :],
                       start=False, stop=(fc == 1))
                rootP_sb = work.tile([P, DM], f32, name="rootPsb")
                S.copy(out=rootP_sb, in_=rp)
            gk = boutk.rearrange("(r p) cj -> r p cj", r=NC_)
            mT_all = big.tile([P, 2, 8, P], f8, name="mT_all", tag="kst", bufs=3)
            m_all = big.tile([P, 8, 272], f8, name="m_all", tag="vall", bufs=2)
            # In gen (split) the S/V queues are idle during the gather gap and
            # nothing behind these loads on S/V is needed before coll-1 lands,
            # so fan the midT loads over 4 queues: any score-chunk order the
            # scheduler picks then has its region resident ~immediately. In
            # the pipelined known phase S/V carry the previous step's attn
            # work, so only sync/G may block on the collective there.
            mT_eng = ((nc.sync, S, G) if split else (nc.sync, G))
            ne = len(mT_eng)
            for r in range(NC_):
                mT_eng[r % ne].dma_start(out=mT_all[:, :, r, :],
                                         in_=gk[r][:, 0:2 * P].rearrange(
                                             "p (c j) -> p c j", j=P))
            gm = (bout2 if split else boutk).rearrange(
                "(r p) cj -> r p cj", r=NC_)
            moff = 0 if split else 2 * P
            # Interleave by parity so the first-consumed regions (0,1) land
            # in parallel rather than serialized on one queue.
            for r in range(NC_):
                ke = nc.sync if r % 2 == 0 else G
                ke.dma_start(out=m_all[:, r, 0:DM],
                             in_=gm[r][:, moff:moff + DM])
            V.memset(m_all[:, :, DM:DM + 1], 1.0)
            return (midT, mT_all, m_all, M1sb, brpe, idx, mask_v, rootP_sb)

        def tconv1_finish(midT, mT_all, m_all, M1sb, brpe, idx, mask_v=False,
                          rootP_sb=None):
            return tconv1_attn(midT, mT_all, m_all, M1sb, brpe, idx, mask_v,
                               rootP_sb)

        hT_last = None
        h_nm_last = None
        pending = []
        for i in range(N_KNOWN):
            kxTa = work.tile([AUG, N], f16, name="kxTa", bufs=4)
            nc.sync.dma_start(out=kxTa, in_=d["kxTaug_all"][i])
            kxa = work.tile([P, 8, 32], f8, name="kxa", bufs=4)
            nc.sync.dma_start(out=kxa, in_=d["kxaug_all"][i])
            kxTl = work.tile([AUG, P], f16, name="kxTl", bufs=4)
            nc.sync.dma_start(out=kxTl, in_=d["kxTaug_loc"][i])
            midT, midT8, m8 = tconv0(
                lambda cc, _t=kxTa: _t[:, cc * P:(cc + 1) * P],
                kxa, kxTl, mask_v=True)
            pending.append(tconv1_stage(midT, midT8, m8, i, mask_v=True))
            if len(pending) > 2:
                st = pending.pop(0)
                h_nm_last, hT_last = tconv1_finish(*st)
                cache_update(st[5], hT_last)
        for st in pending:
            h_nm_last, hT_last = tconv1_finish(*st)
            cache_update(st[5], hT_last)

        # ------------------------------------------------------------------
        # Phase G: autoregressive generation
        # ------------------------------------------------------------------
        for t in range(K, K + N_GEN):
            xn32 = enc(t, h_nm_last)
            if t == K:
                llb = d["latlon32"]
                ll10 = bass.AP(tensor=llb.tensor, offset=llb.offset,
                               ap=[[0, N_GEN]] + [list(a) for a in llb.ap])
                nc.scalar.dma_start(out=out_d[:, :, 0:2], in_=ll10)
            nc.scalar.dma_start(out=out_d[t - K, :, 2:F], in_=xn32)
            if t == K + N_GEN - 1 or t == NG - 1:
                break
            tp = ps_m([F - 2, P], "ptp", dtype=f16)
            mm(tp, xaug[:, 0:F - 2], idn, start=True, stop=True,
               is_transpose=True)
            V.tensor_copy(out=xTaug[0:F - 2, :], in_=tp)
            gin = dram.tile([1, 2 * AUG * P], f16, name="g_in")
            gout = dram.tile([NC_, 2 * AUG * P], f16, name="g_out",
                             addr_space="Shared")
            nc.sync.dma_start(
                out=gin[0, 0:AUG * P].rearrange("(p j) -> p j", p=AUG), in_=xTaug)
            G.dma_start(
                out=gin[0, AUG * P:2 * AUG * P].rearrange("(p j) -> p j", p=P),
                in_=xaug)
            nc.gpsimd.collective_compute(
                "AllGather", OP.bypass, replica_groups=RG,
                ins=[gin[:]], outs=[gout[:]])
            xTa_all = work.tile([AUG, 8, P], f16, name="xTa_all")
            nc.sync.dma_start(
                out=xTa_all,
                in_=gout[:, 0:AUG * P].rearrange("r (p j) -> p r j", p=AUG))
            xa_all = work.tile([P, 8, AUG], f16, name="xa_all")
            G.dma_start(
                out=xa_all,
                in_=gout[:, AUG * P:2 * AUG * P].rearrange("r (p j) -> p r j", p=P))
            V.tensor_copy(out=xa8_st[:, :, 0:AUG], in_=xa_all)
            midT, midT8, m8 = tconv0(lambda cc, _t=xTa_all: _t[:, cc, :],
                                     xa8_st, xTaug, mask_v=True)
            st = tconv1_stage(midT, midT8, m8, t, mask_v=True, early_root=True,
                              split=True)
            h_nm_last, hT_last = tconv1_finish(*st)
            cache_update(t, hT_last)

    nc.finalize()
    return nc


# ----------------------------------------------------------------------------
# Host-side preprocessing
# ----------------------------------------------------------------------------
def prep_in_maps(inputs):
    import ml_dtypes
    bf16np = ml_dtypes.bfloat16
    f8np = ml_dtypes.float8_e4m3
    f32 = np.float32
    f16 = np.float16
    g = {k: np.asarray(v) for k, v in inputs.items()}
    kx = g["known_x"].astype(f32)                       # [10, 1024, 10]
    ei = g["edge_index"].astype(np.int64)

    Cnt = np.zeros((N, N), f32)
    np.add.at(Cnt, (ei[0], ei[1]), 1.0)
    LT = np.where(Cnt > 0, np.log(np.maximum(Cnt, 1.0)), _NEG).astype(f32)

    isd = f32(1.0 / np.sqrt(D))
    PERM = [2, 3, 4, 5, 6, 7, 8, 9, 0, 1, 10]
    Wq0a = (np.vstack([g["Wq0"], g["bq0"][None]]).astype(f32) * isd)[PERM]
    Wk0a = np.vstack([g["Wk0"], g["bk0"][None]]).astype(f32)[PERM]
    A = np.stack([(Wk0a[:, h * D:(h + 1) * D] @ Wq0a[:, h * D:(h + 1) * D].T)
                  for h in range(H)])                                # [4, 11, 11]
    AT = A.transpose(0, 2, 1).transpose(1, 0, 2).copy()              # [11, 4, 11]

    kxaug = np.concatenate([kx, np.ones((K, N, 1), f32)], axis=2)[:, :, PERM]
    kxTaug = kxaug.transpose(0, 2, 1).copy()                         # [10, 11, 1024]

    ide = f32(1.0 / np.sqrt(DH))
    Wqkv, bqkv = g["Wqkv"].astype(f32), g["bqkv"].astype(f32)

    def w2t(w, nch):
        m = w.shape[1]
        return np.ascontiguousarray(
            np.asarray(w, f32).reshape(nch, P, m).transpose(1, 0, 2))

    Wv0a = np.vstack([g["Wv0"], g["bv0"][None]]).astype(f32)[PERM]   # [11, 1024]
    Wv0az = np.zeros((AUG, H, D + 1), f32)
    for h in range(H):
        Wv0az[:, h, 0:D] = Wv0a[:, h * D:(h + 1) * D]
    Wv0az[10, :, D] = 1.0      # e10 column (ones-row index under PERM) -> z

    # Layer-1 bilinear score form: s[dst,src] = u_dst @ Ahat_h @ mid_src^T
    # with u_dst = [mid_dst, 1]; per-dst-constant terms (q.bk) cancel in the
    # softmax over src, so the k-bias column is dropped.
    Wq1a = np.vstack([np.asarray(g["Wq1"], f32),
                      np.asarray(g["bq1"], f32)[None]]) * isd    # [257, 1024]
    Wk1f = np.asarray(g["Wk1"], f32)                             # [256, 1024]
    Ahat = np.stack([Wq1a[:, h * D:(h + 1) * D]
                     @ Wk1f[:, h * D:(h + 1) * D].T
                     for h in range(H)])                         # [H, 257, 256]
    Ahat1 = np.ascontiguousarray(
        Ahat[:, 0:256, :].reshape(H, 2, P, DM).transpose(2, 1, 0, 3))
    # AhatBt[c', cb, h] = Ahat_h[256, cb*128 + c'] (bq-row, added post-matmul)
    AhatBt = np.ascontiguousarray(
        Ahat[:, 256, :].reshape(H, 2, P).transpose(2, 1, 0))

    Wdp = np.asarray(g["g2"], f32)[:, None] * np.asarray(g["Wd"], f32)
    W2d8 = np.asarray(g["W2"], f32) @ Wdp                      # [2048, 8]
    c1v = np.asarray(g["b2f"], f32) @ Wdp                      # [8]
    negSv = -Wdp.sum(axis=0)                                   # [8]
    c2v = (np.asarray(g["bd"], f32)
           + np.asarray(g["be2"], f32) @ np.asarray(g["Wd"], f32))

    common = {
        "AT": AT.astype(f16),
        "Wv0az": Wv0az.astype(f16),
        "Wr0a": np.vstack([g["Wr0"], g["br0"][None]])[PERM].astype(f16),
        "Ahat1": Ahat1.astype(f16),
        "Ahat18": Ahat1.astype(f8np),
        "AhatBt": AhatBt.astype(f16),
        "Wv1": w2t(g["Wv1"], 2).astype(f16),
        "Wr1": w2t(g["Wr1"], 2).astype(f16),
        "brpe1": (np.asarray(g["br1"], f32)[None]
                  + np.asarray(g["bv1"], f32).reshape(H, D).mean(axis=0)[None]
                  + np.asarray(g["pe"], f32))[:, None, :].astype(f16),
        "Wqe": w2t(Wqkv[:, 0:DM] * ide, 2).astype(f16),
        "Wke": w2t(Wqkv[:, DM:2 * DM], 2).astype(f16),
        "Wve": w2t(Wqkv[:, 2 * DM:], 2).astype(f16),
        "bqe": (bqkv[0:DM] * ide)[None].astype(f16),
        "bke": bqkv[DM:2 * DM][None].astype(f16),
        "bve": bqkv[2 * DM:][None].astype(f16),
        "Wo": w2t(g["Wo"], 2).astype(f16),
        "bo": np.asarray(g["bo"], f16)[None],
        "W1": w2t(g["W1"], 2).astype(f16),
        "b1f": np.asarray(g["b1f"], f16)[None],
        "b1fc": np.ascontiguousarray(
            np.asarray(g["b1f"], f32).reshape(16, P).T),
        "W2": w2t(g["W2"], 16).astype(f16),
        "b2f": np.asarray(g["b2f"], f16)[None],
        "Wd": w2t(Wdp, 2).astype(f16),
        "W2d": w2t(W2d8, 16).astype(f16),
        "c1r": c1v[None].astype(f16),
        "nsB1": negSv[None].astype(f32),
        "bdB1": c2v[None].astype(f32),
        "bd": np.asarray(g["bd"], f16)[None],
        "g1": np.asarray(g["g1"], f32)[None],
        "be1": np.asarray(g["be1"], f32)[None],
        "g2": np.asarray(g["g2"], f32)[None],
        "be2": np.asarray(g["be2"], f32)[None],
        "idn": np.eye(P, dtype=f16),
        "ones1": np.ones((1, P), f16),
        "kxTaug_all": kxTaug.astype(f16),
        "kxaug_all": np.ascontiguousarray(
            np.concatenate([kxaug, np.zeros((K, N, 32 - AUG), f32)], axis=2)
            .reshape(K, 8, P, 32).transpose(0, 2, 1, 3)).astype(f8np),
    }
    in_maps = []
    for c in range(NC_):
        sl = slice(P * c, P * (c + 1))
        m = dict(common)
        m["lt"] = np.ascontiguousarray(
            LT[:, sl].reshape(8, P, P).transpose(1, 0, 2)).astype(f16)
        m["cnt"] = np.ascontiguousarray(
            Cnt[:, sl].reshape(8, P, P).transpose(1, 0, 2)).astype(f16)
        m["latlon32"] = np.ascontiguousarray(kx[K - 1, sl, 0:2]).astype(f32)
        ll = kx[K - 1, sl, 0:2].astype(f32)
        xti = np.zeros((AUG, P), f32); xti[8:10] = ll.T; xti[10] = 1.0
        m["xTaug_init"] = xti.astype(f16)
        xai = np.zeros((P, AUG), f32); xai[:, 8:10] = ll; xai[:, 10] = 1.0
        m["xaug_init"] = xai.astype(f16)
        m["kxTaug_loc"] = np.ascontiguousarray(kxTaug[:, :, sl]).astype(f16)
        in_maps.append(m)
    return in_maps


_CACHED = {}


def run(inputs, trace=False, trace_kwargs=None):
    from concourse import bass_utils
    if "nc" not in _CACHED:
        _CACHED["nc"] = build_bass()
    in_maps = prep_in_maps(inputs)
    res = bass_utils.run_bass_kernel_spmd(
        _CACHED["nc"], in_maps, core_ids=list(range(NC_)), trace=trace,
        **(trace_kwargs or {}))
    out = np.concatenate([res.results[c]["out"] for c in range(NC_)], axis=1)
    return out.astype(np.float32), res


def kernel(**inputs):
    out, _ = run(inputs, trace=False)
    return out

